# revision 2
# baseline (speedup 1.0000x reference)
"""GRU (5-layer, H=128) Trainium2 Bass kernel.

Strategy: pure data parallel over batch (64 / 8 cores = 8 per core).
Per core, the 5 layers run as a chunk-staggered wavefront (chunk C=16
timesteps): layer l processes chunk (m - l) during "round" m.  All
per-step elementwise work for the 5 active layers is batched into
wide ops on [128, nl*8] tiles.

Perf structure:
  - The per-step bottleneck is the tensor engine's 15 LDWEIGHTS
    (5 layers x 3 gates) per recurrence step.  The hidden-to-hidden
    weights are stored fp8 (e3m4) so FWL streams them 4B/cycle
    (~27ns vs ~53ns bf16); the moving operand (h) stays bf16
    (mixed-dtype matmul).  All gate weights/biases are pre-scaled by
    2^7 so fp8 hits its normal range; the scale is unwound for free
    via the activation `scale` operand on sigmoid/tanh.
  - Input-to-hidden gates are precomputed per chunk directly into
    PSUM (bias via rank<=4 ones-trick matmuls, then ih matmuls);
    ih weights stay bf16 (x128) -- accuracy, and their LDWEIGHTS
    hide under the larger N=128 matmuls.
  - x is pre-transposed on the host ([IN,T,B] layout), removing the
    per-chunk PE transpose + copy of the old design.
  - Elementwise chain per step (batched over active layers),
    critical path 6 ops:
      sig -> r*hn -> +xn -> tanh -> (z-1)*n [stt] -> p - (z-1)*n
    with p = z*h computed off-path on GPSIMD.

Layouts (per core, SBUF):
  hW  [128, 5, 577, 8]  h history; slot W stores h_l(t) at W = t + 16*l + 1
                        (W = 16l holds the per-layer zero initial state)
  xc  [128, C, 4, 8]    current input chunk (DMA'd from host-transposed x)
  P_all (PSUM) [128, 3, 5, 16, 8]  r/z/xn pre-activations for one chunk
  P_hn  (PSUM) [128, 5, 16, 8]     W_hn h + b_hhn for one chunk
"""

import sys

for p in ("/opt/trn_rl_repo", "/opt/pypackages"):
    if p not in sys.path:
        sys.path.append(p)

import numpy as np
import ml_dtypes

BFNP = ml_dtypes.bfloat16
F8NP = ml_dtypes.float8_e3m4

import concourse.bass as bass  # noqa: F401
import concourse.mybir as mybir
import concourse.tile as tile
from concourse import bacc

F32 = mybir.dt.float32
BF16 = mybir.dt.bfloat16
FP8 = mybir.dt.float8e3
AF = mybir.ActivationFunctionType
ALU = mybir.AluOpType

H = 128
L = 5
NCORE = 8
BC = 8  # batch per core
IN = 512
OUT = 96
C = 16  # chunk (timesteps)

WSCALE = 128.0  # gate weights/biases pre-scaled by this; unwound in ACT
INV_WSCALE = 1.0 / WSCALE


def build_nc(T=512):
    NCH = T // C
    NR = NCH + L - 1
    WDIM = T + C * (L - 1) + 1

    nc = bacc.Bacc("TRN2", target_bir_lowering=False, debug=False)

    xT = nc.dram_tensor("xT", [H, T, 4, BC], BF16, kind="ExternalInput")
    whhT = nc.dram_tensor("whhT", [H, L, 3, H], FP8, kind="ExternalInput")
    wihT = nc.dram_tensor("wihT", [H, L - 1, 3, H], BF16, kind="ExternalInput")
    wih0T = nc.dram_tensor("wih0T", [H, 4, 3, H], BF16, kind="ExternalInput")
    fcT = nc.dram_tensor("fcT", [H, OUT], BF16, kind="ExternalInput")
    biasP = nc.dram_tensor("biasP", [4, 4, H], BF16, kind="ExternalInput")
    oh4 = nc.dram_tensor("oh4", [4, 512], BF16, kind="ExternalInput")
    bhn = nc.dram_tensor("bhn", [4, H], BF16, kind="ExternalInput")
    bl4 = nc.dram_tensor("bl4", [1, H], BF16, kind="ExternalInput")
    on1 = nc.dram_tensor("on1", [1, H], BF16, kind="ExternalInput")
    fcb = nc.dram_tensor("fcb", [BC, OUT], F32, kind="ExternalInput")
    y = nc.dram_tensor("y", [BC, OUT], F32, kind="ExternalOutput")

    with tile.TileContext(nc) as tc:
        with (
            tc.tile_pool(name="persist", bufs=1) as pp,
            tc.tile_pool(name="xsrc", bufs=3) as xsp,
            tc.tile_pool(name="tmp", bufs=3) as tp,
            tc.tile_pool(name="pall", bufs=1, space="PSUM") as pallp,
            tc.tile_pool(name="phn", bufs=1, space="PSUM") as phnp,
            tc.tile_pool(name="pfc", bufs=1, space="PSUM") as pfcp,
        ):
            hW = pp.tile([H, L, WDIM, BC], BF16, tag="hW")
            whh_sb = pp.tile([H, L, 3, H], FP8, tag="whh")
            wih_sb = pp.tile([H, L - 1, 3, H], BF16, tag="wih")
            wih0_sb = pp.tile([H, 4, 3, H], BF16, tag="wih0")
            fcT_sb = pp.tile([H, OUT], BF16, tag="fcT")
            biasP_sb = pp.tile([4, 4, H], BF16, tag="biasP")
            oh4_sb = pp.tile([4, 512], BF16, tag="oh4")
            bhn_sb = pp.tile([4, H], BF16, tag="bhn")
            bl4_sb = pp.tile([1, H], BF16, tag="bl4")
            on1_sb = pp.tile([1, H], BF16, tag="on1")
            fcb_sb = pp.tile([BC, OUT], F32, tag="fcb")

            nc.sync.dma_start(whh_sb[:, :, :, :], whhT[:, :, :, :])
            nc.sync.dma_start(wih0_sb[:, :, :, :], wih0T[:, :, :, :])
            nc.sync.dma_start(wih_sb[:, :, :, :], wihT[:, :, :, :])
            nc.sync.dma_start(biasP_sb[:, :, :], biasP[:, :, :])
            nc.sync.dma_start(oh4_sb[:, :], oh4[:, :])
            nc.sync.dma_start(bhn_sb[:, :], bhn[:, :])
            nc.sync.dma_start(bl4_sb[:, :], bl4[:, :])
            nc.sync.dma_start(on1_sb[:, :], on1[:, :])
            nc.sync.dma_start(fcT_sb[:, :], fcT[:, :])
            nc.sync.dma_start(fcb_sb[:, :], fcb[:, :])

            for l in range(L):
                nc.vector.memset(hW[:, l, C * l, :], 0.0)

            def load_x_chunk(m):
                xc = xsp.tile([H, C, 4, BC], BF16, tag="xc")
                nc.sync.dma_start(xc[:, :, :, :], xT[:, m * C : (m + 1) * C, :, :])
                return xc

            xc_cur = load_x_chunk(0)

            for m in range(NR):
                la0 = max(0, m - (NCH - 1))
                la1 = min(L - 1, m)
                sl = slice(la0, la1 + 1)

                P_all = pallp.tile([H, 3, L, C, BC], F32, tag="P_all")
                P_hn = phnp.tile([H, L, C, BC], F32, tag="P_hn")
                Pfl = P_all[:, :, :, :, :].rearrange("p g l c b -> p (g l c b)")
                Phfl = P_hn[:, :, :, :].rearrange("p l c b -> p (l c b)")

                # bias accumulation (start=True) via rank<=4 ones-trick
                for bk in range(4):
                    kk = 4 if bk < 3 else 3
                    N = 512 if bk < 3 else 384
                    nc.tensor.matmul(
                        Pfl[:, bk * 512 : bk * 512 + N],
                        biasP_sb[0:kk, bk, :],
                        oh4_sb[0:kk, 0:N],
                        start=True,
                        stop=False,
                        skip_group_check=True,
                    )
                nc.tensor.matmul(
                    Phfl[:, 0:512],
                    bhn_sb[0:4, :],
                    oh4_sb[0:4, 0:512],
                    start=True,
                    stop=False,
                    skip_group_check=True,
                )
                nc.tensor.matmul(
                    Phfl[:, 512:640],
                    bl4_sb[0:1, :],
                    on1_sb[0:1, 0:H],
                    start=True,
                    stop=False,
                    skip_group_check=True,
                )

                # input-gate (ih) chunk matmuls
                if m < NCH:  # layer 0 reads x chunk m
                    for g in range(3):
                        for ki in range(4):
                            nc.tensor.matmul(
                                P_all[:, g, 0, :, :],
                                wih0_sb[:, ki, g, :],
                                xc_cur[:, :, ki, :],
                                start=False,
                                stop=False,
                                skip_group_check=True,
                            )
                for l in range(max(1, la0), la1 + 1):
                    for g in range(3):
                        nc.tensor.matmul(
                            P_all[:, g, l, :, :],
                            wih_sb[:, l - 1, g, :],
                            hW[:, l - 1, C * m - 15 : C * m + 1, :],
                            start=False,
                            stop=False,
                            skip_group_check=True,
                        )

                if m + 1 < NCH:
                    xc_cur = load_x_chunk(m + 1)

                for j in range(C):
                    base = C * m + j
                    # hidden-gate matmuls for this step
                    for l in range(la0, la1 + 1):
                        for g in range(3):
                            dest = (
                                P_all[:, g, l, j, :]
                                if g < 2
                                else P_hn[:, l, j, :]
                            )
                            nc.tensor.matmul(
                                dest,
                                whh_sb[:, l, g, :],
                                hW[:, l, base, :],
                                start=False,
                                stop=True,
                                skip_group_check=True,
                            )
                    # elementwise chain, batched over active layers
                    rzt = tp.tile([H, 2, L, BC], F32, tag="rz")
                    rnt = tp.tile([H, L, BC], F32, tag="rn")
                    npret = tp.tile([H, L, BC], F32, tag="npre")
                    nt = tp.tile([H, L, BC], F32, tag="nt")
                    tneg = tp.tile([H, L, BC], F32, tag="tneg")
                    pt = tp.tile([H, L, BC], F32, tag="pt")
                    nc.scalar.activation(
                        rzt[:, :, sl, :], P_all[:, 0:2, sl, j, :], AF.Sigmoid,
                        scale=INV_WSCALE,
                    )
                    # p = z*h (off critical path)
                    nc.gpsimd.tensor_tensor(
                        pt[:, sl, :], rzt[:, 1, sl, :], hW[:, sl, base, :], ALU.mult
                    )
                    nc.vector.tensor_tensor(
                        rnt[:, sl, :], rzt[:, 0, sl, :], P_hn[:, sl, j, :], ALU.mult
                    )
                    nc.vector.tensor_tensor(
                        npret[:, sl, :], rnt[:, sl, :], P_all[:, 2, sl, j, :], ALU.add
                    )
                    nc.scalar.activation(
                        nt[:, sl, :], npret[:, sl, :], AF.Tanh, scale=INV_WSCALE
                    )
                    # tneg = (z - 1) * n
                    nc.vector.scalar_tensor_tensor(
                        tneg[:, sl, :], rzt[:, 1, sl, :], 1.0, nt[:, sl, :],
                        ALU.subtract, ALU.mult,
                    )
                    # h' = p - tneg = z*h + (1-z)*n
                    nc.vector.tensor_tensor(
                        hW[:, sl, base + 1, :], pt[:, sl, :], tneg[:, sl, :],
                        ALU.subtract,
                    )

            # final FC on last timestep of layer 4
            pfc = pfcp.tile([BC, OUT], F32, tag="fc")
            nc.tensor.matmul(
                pfc[:, :],
                hW[:, L - 1, WDIM - 1, :],
                fcT_sb[:, :],
                start=True,
                stop=True,
                skip_group_check=True,
            )
            out_sb = pp.tile([BC, OUT], F32, tag="out")
            nc.vector.tensor_tensor(out_sb[:, :], pfc[:, :], fcb_sb[:, :], ALU.add)
            nc.sync.dma_start(y[:, :], out_sb[:, :])

    nc.compile()
    return nc


def prep_shared(w_ih0, w_ih_rest, w_hh, b_ih, b_hh, fc_w, fc_b):
    d = {}
    whhT = np.empty([H, L, 3, H], np.float32)
    for l in range(L):
        for g in range(3):
            whhT[:, l, g, :] = w_hh[l, g * H : (g + 1) * H, :].T
    d["whhT"] = (whhT * WSCALE).astype(F8NP)
    wihT = np.empty([H, L - 1, 3, H], np.float32)
    for l in range(1, L):
        for g in range(3):
            wihT[:, l - 1, g, :] = w_ih_rest[l - 1, g * H : (g + 1) * H, :].T
    d["wihT"] = (wihT * WSCALE).astype(BFNP)
    wih0T = np.empty([H, 4, 3, H], np.float32)
    for ki in range(4):
        for g in range(3):
            wih0T[:, ki, g, :] = w_ih0[g * H : (g + 1) * H, ki * H : (ki + 1) * H].T
    d["wih0T"] = (wih0T * WSCALE).astype(BFNP)
    d["fcT"] = np.ascontiguousarray(fc_w.T).astype(BFNP)
    biasP = np.zeros([4, 4, H], np.float32)
    for i in range(15):  # chunk index i = g*5 + l
        g, l = divmod(i, 5)
        b = b_ih[l, g * H : (g + 1) * H].astype(np.float32)
        if g < 2:
            b = b + b_hh[l, g * H : (g + 1) * H]
        biasP[i % 4, i // 4, :] = b  # dims: [k, bank, H]
    d["biasP"] = (biasP * WSCALE).astype(BFNP)
    oh4 = np.zeros([4, 512], np.float32)
    for k in range(4):
        oh4[k, k * H : (k + 1) * H] = 1.0
    d["oh4"] = oh4.astype(BFNP)
    d["bhn"] = (b_hh[0:4, 2 * H : 3 * H].astype(np.float32) * WSCALE).astype(BFNP)
    d["bl4"] = (b_hh[4:5, 2 * H : 3 * H].astype(np.float32) * WSCALE).astype(BFNP)
    d["on1"] = np.ones([1, H], np.float32).astype(BFNP)
    d["fcb"] = np.tile(fc_b.astype(np.float32)[None, :], (BC, 1))
    return d


_NC_CACHE = {}


def run(x, w_ih0, w_ih_rest, w_hh, b_ih, b_hh, fc_w, fc_b, T=512, **run_kwargs):
    from concourse.bass_utils import run_bass_kernel_spmd

    if T not in _NC_CACHE:
        _NC_CACHE[T] = build_nc(T)
    nc = _NC_CACHE[T]
    shared = prep_shared(
        np.asarray(w_ih0), np.asarray(w_ih_rest), np.asarray(w_hh),
        np.asarray(b_ih), np.asarray(b_hh), np.asarray(fc_w), np.asarray(fc_b),
    )
    x = np.asarray(x)
    in_maps = []
    for c in range(NCORE):
        m = dict(shared)
        m["xT"] = prep_x(x, c, T)
        in_maps.append(m)
    res = run_bass_kernel_spmd(nc, in_maps, core_ids=list(range(NCORE)), **run_kwargs)
    out = np.concatenate([res.results[c]["y"] for c in range(NCORE)], axis=0)
    return out, res


def prep_x(x, c, T):
    xs = x[c * BC : (c + 1) * BC, :T, :]  # [BC, T, IN]
    # -> [H, T, 4, BC]: element [p, t, ki, b] = x[b, t, ki*128+p]
    xt = xs.transpose(2, 1, 0).reshape(4, H, T, BC)  # [ki, p, t, b]
    return np.ascontiguousarray(xt.transpose(1, 2, 0, 3)).astype(BFNP)


def kernel(x, w_ih0, w_ih_rest, w_hh, b_ih, b_hh, fc_w, fc_b):
    out, _ = run(x, w_ih0, w_ih_rest, w_hh, b_ih, b_hh, fc_w, fc_b, T=512)
    return out.astype(np.float32)


if __name__ == "__main__":
    # quick smoke test at small T against a numpy reference
    T = int(sys.argv[1]) if len(sys.argv) > 1 else 64
    rng = np.random.default_rng(0)
    s = 1.0 / np.sqrt(H)
    u = lambda *sh: rng.uniform(-s, s, sh).astype(np.float32)
    x = rng.standard_normal((64, T, IN), dtype=np.float32)
    w_ih0 = u(3 * H, IN)
    w_ih_rest = u(L - 1, 3 * H, H)
    w_hh = u(L, 3 * H, H)
    b_ih = u(L, 3 * H)
    b_hh = u(L, 3 * H)
    fc_w = u(OUT, H)
    fc_b = u(OUT)

    def np_ref():
        sig = lambda v: 1.0 / (1.0 + np.exp(-v))
        h_in = x.astype(np.float64)
        for l in range(L):
            wi = (w_ih0 if l == 0 else w_ih_rest[l - 1]).astype(np.float64)
            wh = w_hh[l].astype(np.float64)
            gx = np.einsum("bti,gi->btg", h_in, wi) + b_ih[l]
            h = np.zeros((64, H))
            hs = []
            for t in range(T):
                gh = h @ wh.T + b_hh[l]
                xr, xz, xn = np.split(gx[:, t], 3, -1)
                hr, hz, hn = np.split(gh, 3, -1)
                r = sig(xr + hr)
                z = sig(xz + hz)
                n = np.tanh(xn + r * hn)
                h = (1 - z) * n + z * h
                hs.append(h)
            h_in = np.stack(hs, 1)
        return h_in[:, -1] @ fc_w.astype(np.float64).T + fc_b

    exp = np_ref()
    got, res = run(x, w_ih0, w_ih_rest, w_hh, b_ih, b_hh, fc_w, fc_b, T=T)
    err = np.abs(got - exp)
    rel = np.linalg.norm(got - exp) / np.linalg.norm(exp)
    print("max abs err:", err.max(), "rel:", rel)
    print("exec_time_ns:", res.exec_time_ns)


# revision 10
# speedup vs baseline: 3.1933x; 3.1933x over previous
"""GRU (5-layer, H=128) Trainium2 Bass kernel.

Strategy: pure data parallel over batch (64 / 8 cores = 8 per core).
Per core, the 5 layers run as a chunk-staggered wavefront (chunk C=16
timesteps): layer l processes chunk (m - l) during "round" m.  All
per-step elementwise work for the 5 active layers is batched into
wide ops on [128, nl*8] tiles.

Perf structure:
  - The per-step bottleneck is the tensor engine's 15 LDWEIGHTS
    (5 layers x 3 gates) per recurrence step.  The hidden-to-hidden
    weights are stored fp8 (e3m4) so FWL streams them 4B/cycle
    (~27ns vs ~53ns bf16); the moving operand (h) stays bf16
    (mixed-dtype matmul).  All gate weights/biases are pre-scaled by
    2^7 so fp8 hits its normal range; the scale is unwound for free
    via the activation `scale` operand on sigmoid/tanh.
  - Input-to-hidden gates are precomputed per chunk directly into
    PSUM (bias via rank<=4 ones-trick matmuls, then ih matmuls);
    ih weights stay bf16 (x128) -- accuracy, and their LDWEIGHTS
    hide under the larger N=128 matmuls.
  - x is pre-transposed on the host ([IN,T,B] layout), removing the
    per-chunk PE transpose + copy of the old design.
  - Elementwise chain per step (batched over active layers),
    critical path 6 ops:
      sig -> r*hn -> +xn -> tanh -> (z-1)*n [stt] -> p - (z-1)*n
    with p = z*h computed off-path on GPSIMD.

Layouts (per core, SBUF):
  hW  [128, 5, 577, 8]  h history; slot W stores h_l(t) at W = t + 16*l + 1
                        (W = 16l holds the per-layer zero initial state)
  xc  [128, C, 4, 8]    current input chunk (DMA'd from host-transposed x)
  P_all (PSUM) [128, 3, 5, 16, 8]  r/z/xn pre-activations for one chunk
  P_hn  (PSUM) [128, 5, 16, 8]     W_hn h + b_hhn for one chunk
"""

import os
import sys

for p in ("/opt/trn_rl_repo", "/opt/pypackages"):
    if p not in sys.path:
        sys.path.append(p)

import numpy as np
import ml_dtypes

# timing-ablation probes (GRU_PROBE env): "nochain" drops the elementwise
# chain (PE free-runs -> PE throughput floor); "nope" drops the per-step
# hh matmuls instead. Output is numerically invalid; timing-only.
PROBE = os.environ.get("GRU_PROBE", "")

BFNP = ml_dtypes.bfloat16
F8NP = ml_dtypes.float8_e3m4

import concourse.bass as bass  # noqa: F401
import concourse.mybir as mybir
import concourse.tile as tile
from concourse import bacc

F32 = mybir.dt.float32
BF16 = mybir.dt.bfloat16
FP8 = mybir.dt.float8e3
AF = mybir.ActivationFunctionType
ALU = mybir.AluOpType

H = 128
L = 5
NCORE = 8
BC = 8  # batch per core
IN = 512
OUT = 96
C = 16  # chunk (timesteps)

WSCALE = 128.0  # gate weights/biases pre-scaled by this; unwound in ACT
INV_WSCALE = 1.0 / WSCALE


def build_nc(T=512):
    NCH = T // C
    NR = NCH + L - 1
    WDIM = T + C * (L - 1) + 1

    nc = bacc.Bacc("TRN2", target_bir_lowering=False, debug=False)

    xT = nc.dram_tensor("xT", [H, T, 4, BC], BF16, kind="ExternalInput")
    whhT = nc.dram_tensor("whhT", [H, L, 3, H], FP8, kind="ExternalInput")
    wihT = nc.dram_tensor("wihT", [H, L - 1, 3, H], BF16, kind="ExternalInput")
    wih0T = nc.dram_tensor("wih0T", [H, 4, 3, H], BF16, kind="ExternalInput")
    fcT = nc.dram_tensor("fcT", [H, OUT], BF16, kind="ExternalInput")
    biasP = nc.dram_tensor("biasP", [4, 4, H], BF16, kind="ExternalInput")
    oh4 = nc.dram_tensor("oh4", [4, 512], BF16, kind="ExternalInput")
    bhn = nc.dram_tensor("bhn", [4, H], BF16, kind="ExternalInput")
    bl4 = nc.dram_tensor("bl4", [1, H], BF16, kind="ExternalInput")
    on1 = nc.dram_tensor("on1", [1, H], BF16, kind="ExternalInput")
    fcb = nc.dram_tensor("fcb", [BC, OUT], F32, kind="ExternalInput")
    y = nc.dram_tensor("y", [BC, OUT], F32, kind="ExternalOutput")

    with tile.TileContext(nc) as tc:
        with (
            tc.tile_pool(name="persist", bufs=1) as pp,
            tc.tile_pool(name="xsrc", bufs=3) as xsp,
            tc.tile_pool(name="tmp", bufs=3) as tp,
            tc.tile_pool(name="pall", bufs=1, space="PSUM") as pallp,
            tc.tile_pool(name="phn", bufs=1, space="PSUM") as phnp,
            tc.tile_pool(name="pfc", bufs=1, space="PSUM") as pfcp,
        ):
            hW = pp.tile([H, L, WDIM, BC], BF16, tag="hW")
            whh_sb = pp.tile([H, L, 3, H], FP8, tag="whh")
            wih_sb = pp.tile([H, L - 1, 3, H], BF16, tag="wih")
            wih0_sb = pp.tile([H, 4, 3, H], BF16, tag="wih0")
            fcT_sb = pp.tile([H, OUT], BF16, tag="fcT")
            biasP_sb = pp.tile([4, 4, H], BF16, tag="biasP")
            oh4_sb = pp.tile([4, 512], BF16, tag="oh4")
            bhn_sb = pp.tile([4, H], BF16, tag="bhn")
            bl4_sb = pp.tile([1, H], BF16, tag="bl4")
            on1_sb = pp.tile([1, H], BF16, tag="on1")
            fcb_sb = pp.tile([BC, OUT], F32, tag="fcb")

            nc.sync.dma_start(whh_sb[:, :, :, :], whhT[:, :, :, :])
            nc.sync.dma_start(wih0_sb[:, :, :, :], wih0T[:, :, :, :])
            nc.sync.dma_start(wih_sb[:, :, :, :], wihT[:, :, :, :])
            nc.sync.dma_start(biasP_sb[:, :, :], biasP[:, :, :])
            nc.sync.dma_start(oh4_sb[:, :], oh4[:, :])
            nc.sync.dma_start(bhn_sb[:, :], bhn[:, :])
            nc.sync.dma_start(bl4_sb[:, :], bl4[:, :])
            nc.sync.dma_start(on1_sb[:, :], on1[:, :])
            nc.sync.dma_start(fcT_sb[:, :], fcT[:, :])
            nc.sync.dma_start(fcb_sb[:, :], fcb[:, :])

            for l in range(L):
                nc.vector.memset(hW[:, l, C * l, :], 0.0)

            def load_x_chunk(m):
                xc = xsp.tile([H, C, 4, BC], BF16, tag="xc")
                nc.sync.dma_start(xc[:, :, :, :], xT[:, m * C : (m + 1) * C, :, :])
                return xc

            xc_cur = load_x_chunk(0)

            for m in range(NR):
                la0 = max(0, m - (NCH - 1))
                la1 = min(L - 1, m)
                sl = slice(la0, la1 + 1)

                P_all = pallp.tile([H, 3, L, C, BC], F32, tag="P_all")
                P_hn = phnp.tile([H, L, C, BC], F32, tag="P_hn")
                Pfl = P_all[:, :, :, :, :].rearrange("p g l c b -> p (g l c b)")
                Phfl = P_hn[:, :, :, :].rearrange("p l c b -> p (l c b)")

                # bias accumulation (start=True) via rank<=4 ones-trick
                for bk in range(4):
                    kk = 4 if bk < 3 else 3
                    N = 512 if bk < 3 else 384
                    nc.tensor.matmul(
                        Pfl[:, bk * 512 : bk * 512 + N],
                        biasP_sb[0:kk, bk, :],
                        oh4_sb[0:kk, 0:N],
                        start=True,
                        stop=False,
                        skip_group_check=True,
                    )
                nc.tensor.matmul(
                    Phfl[:, 0:512],
                    bhn_sb[0:4, :],
                    oh4_sb[0:4, 0:512],
                    start=True,
                    stop=False,
                    skip_group_check=True,
                )
                nc.tensor.matmul(
                    Phfl[:, 512:640],
                    bl4_sb[0:1, :],
                    on1_sb[0:1, 0:H],
                    start=True,
                    stop=False,
                    skip_group_check=True,
                )

                # input-gate (ih) chunk matmuls
                if m < NCH:  # layer 0 reads x chunk m
                    for g in range(3):
                        for ki in range(4):
                            nc.tensor.matmul(
                                P_all[:, g, 0, :, :],
                                wih0_sb[:, ki, g, :],
                                xc_cur[:, :, ki, :],
                                start=False,
                                stop=False,
                                skip_group_check=True,
                            )
                for l in range(max(1, la0), la1 + 1):
                    for g in range(3):
                        nc.tensor.matmul(
                            P_all[:, g, l, :, :],
                            wih_sb[:, l - 1, g, :],
                            hW[:, l - 1, C * m - 15 : C * m + 1, :],
                            start=False,
                            stop=False,
                            skip_group_check=True,
                        )

                if m + 1 < NCH:
                    xc_cur = load_x_chunk(m + 1)

                for j in range(C):
                    base = C * m + j
                    # hidden-gate matmuls for this step; gate-major order so
                    # the r/z pre-acts (sigmoid operands) complete first
                    if PROBE != "nope":
                        for g in range(3):
                            for l in range(la0, la1 + 1):
                                dest = (
                                    P_all[:, g, l, j, :]
                                    if g < 2
                                    else P_hn[:, l, j, :]
                                )
                                nc.tensor.matmul(
                                    dest,
                                    whh_sb[:, l, g, :],
                                    hW[:, l, base, :],
                                    start=False,
                                    stop=True,
                                    skip_group_check=True,
                                )
                    if PROBE == "nochain":
                        continue
                    # elementwise chain, batched over active layers; bf16
                    # intermediates let the pure-SBUF DVE ops run in 2x mode
                    rzt = tp.tile([H, 2, L, BC], BF16, tag="rz")
                    rnt = tp.tile([H, L, BC], F32, tag="rn")
                    npret = tp.tile([H, L, BC], F32, tag="npre")
                    nt = tp.tile([H, L, BC], BF16, tag="nt")
                    tneg = tp.tile([H, L, BC], BF16, tag="tneg")
                    pt = tp.tile([H, L, BC], BF16, tag="pt")
                    nc.scalar.activation(
                        rzt[:, :, sl, :], P_all[:, 0:2, sl, j, :], AF.Sigmoid,
                        scale=INV_WSCALE,
                    )
                    # p = z*h (off critical path)
                    nc.gpsimd.tensor_tensor(
                        pt[:, sl, :], rzt[:, 1, sl, :], hW[:, sl, base, :], ALU.mult
                    )
                    nc.vector.tensor_tensor(
                        rnt[:, sl, :], rzt[:, 0, sl, :], P_hn[:, sl, j, :], ALU.mult
                    )
                    nc.vector.tensor_tensor(
                        npret[:, sl, :], rnt[:, sl, :], P_all[:, 2, sl, j, :], ALU.add
                    )
                    nc.scalar.activation(
                        nt[:, sl, :], npret[:, sl, :], AF.Tanh, scale=INV_WSCALE
                    )
                    # tneg = (z - 1) * n
                    nc.vector.scalar_tensor_tensor(
                        tneg[:, sl, :], rzt[:, 1, sl, :], 1.0, nt[:, sl, :],
                        ALU.subtract, ALU.mult,
                    )
                    # h' = p - tneg = z*h + (1-z)*n
                    nc.vector.tensor_tensor(
                        hW[:, sl, base + 1, :], pt[:, sl, :], tneg[:, sl, :],
                        ALU.subtract,
                    )

            # final FC on last timestep of layer 4
            pfc = pfcp.tile([BC, OUT], F32, tag="fc")
            nc.tensor.matmul(
                pfc[:, :],
                hW[:, L - 1, WDIM - 1, :],
                fcT_sb[:, :],
                start=True,
                stop=True,
                skip_group_check=True,
            )
            out_sb = pp.tile([BC, OUT], F32, tag="out")
            nc.vector.tensor_tensor(out_sb[:, :], pfc[:, :], fcb_sb[:, :], ALU.add)
            nc.sync.dma_start(y[:, :], out_sb[:, :])

    nc.compile()
    return nc


def prep_shared(w_ih0, w_ih_rest, w_hh, b_ih, b_hh, fc_w, fc_b):
    d = {}
    whhT = np.empty([H, L, 3, H], np.float32)
    for l in range(L):
        for g in range(3):
            whhT[:, l, g, :] = w_hh[l, g * H : (g + 1) * H, :].T
    d["whhT"] = (whhT * WSCALE).astype(F8NP)
    wihT = np.empty([H, L - 1, 3, H], np.float32)
    for l in range(1, L):
        for g in range(3):
            wihT[:, l - 1, g, :] = w_ih_rest[l - 1, g * H : (g + 1) * H, :].T
    d["wihT"] = (wihT * WSCALE).astype(BFNP)
    wih0T = np.empty([H, 4, 3, H], np.float32)
    for ki in range(4):
        for g in range(3):
            wih0T[:, ki, g, :] = w_ih0[g * H : (g + 1) * H, ki * H : (ki + 1) * H].T
    d["wih0T"] = (wih0T * WSCALE).astype(BFNP)
    d["fcT"] = np.ascontiguousarray(fc_w.T).astype(BFNP)
    biasP = np.zeros([4, 4, H], np.float32)
    for i in range(15):  # chunk index i = g*5 + l
        g, l = divmod(i, 5)
        b = b_ih[l, g * H : (g + 1) * H].astype(np.float32)
        if g < 2:
            b = b + b_hh[l, g * H : (g + 1) * H]
        biasP[i % 4, i // 4, :] = b  # dims: [k, bank, H]
    d["biasP"] = (biasP * WSCALE).astype(BFNP)
    oh4 = np.zeros([4, 512], np.float32)
    for k in range(4):
        oh4[k, k * H : (k + 1) * H] = 1.0
    d["oh4"] = oh4.astype(BFNP)
    d["bhn"] = (b_hh[0:4, 2 * H : 3 * H].astype(np.float32) * WSCALE).astype(BFNP)
    d["bl4"] = (b_hh[4:5, 2 * H : 3 * H].astype(np.float32) * WSCALE).astype(BFNP)
    d["on1"] = np.ones([1, H], np.float32).astype(BFNP)
    d["fcb"] = np.tile(fc_b.astype(np.float32)[None, :], (BC, 1))
    return d


_NC_CACHE = {}


def run(x, w_ih0, w_ih_rest, w_hh, b_ih, b_hh, fc_w, fc_b, T=512, **run_kwargs):
    from concourse.bass_utils import run_bass_kernel_spmd

    if T not in _NC_CACHE:
        _NC_CACHE[T] = build_nc(T)
    nc = _NC_CACHE[T]
    shared = prep_shared(
        np.asarray(w_ih0), np.asarray(w_ih_rest), np.asarray(w_hh),
        np.asarray(b_ih), np.asarray(b_hh), np.asarray(fc_w), np.asarray(fc_b),
    )
    x = np.asarray(x)
    in_maps = []
    for c in range(NCORE):
        m = dict(shared)
        m["xT"] = prep_x(x, c, T)
        in_maps.append(m)
    res = run_bass_kernel_spmd(nc, in_maps, core_ids=list(range(NCORE)), **run_kwargs)
    out = np.concatenate([res.results[c]["y"] for c in range(NCORE)], axis=0)
    return out, res


def prep_x(x, c, T):
    xs = x[c * BC : (c + 1) * BC, :T, :]  # [BC, T, IN]
    # -> [H, T, 4, BC]: element [p, t, ki, b] = x[b, t, ki*128+p]
    xt = xs.transpose(2, 1, 0).reshape(4, H, T, BC)  # [ki, p, t, b]
    return np.ascontiguousarray(xt.transpose(1, 2, 0, 3)).astype(BFNP)


def kernel(x, w_ih0, w_ih_rest, w_hh, b_ih, b_hh, fc_w, fc_b):
    out, _ = run(x, w_ih0, w_ih_rest, w_hh, b_ih, b_hh, fc_w, fc_b, T=512)
    return out.astype(np.float32)


if __name__ == "__main__":
    # quick smoke test at small T against a numpy reference
    T = int(sys.argv[1]) if len(sys.argv) > 1 else 64
    rng = np.random.default_rng(0)
    s = 1.0 / np.sqrt(H)
    u = lambda *sh: rng.uniform(-s, s, sh).astype(np.float32)
    x = rng.standard_normal((64, T, IN), dtype=np.float32)
    w_ih0 = u(3 * H, IN)
    w_ih_rest = u(L - 1, 3 * H, H)
    w_hh = u(L, 3 * H, H)
    b_ih = u(L, 3 * H)
    b_hh = u(L, 3 * H)
    fc_w = u(OUT, H)
    fc_b = u(OUT)

    def np_ref():
        sig = lambda v: 1.0 / (1.0 + np.exp(-v))
        h_in = x.astype(np.float64)
        for l in range(L):
            wi = (w_ih0 if l == 0 else w_ih_rest[l - 1]).astype(np.float64)
            wh = w_hh[l].astype(np.float64)
            gx = np.einsum("bti,gi->btg", h_in, wi) + b_ih[l]
            h = np.zeros((64, H))
            hs = []
            for t in range(T):
                gh = h @ wh.T + b_hh[l]
                xr, xz, xn = np.split(gx[:, t], 3, -1)
                hr, hz, hn = np.split(gh, 3, -1)
                r = sig(xr + hr)
                z = sig(xz + hz)
                n = np.tanh(xn + r * hn)
                h = (1 - z) * n + z * h
                hs.append(h)
            h_in = np.stack(hs, 1)
        return h_in[:, -1] @ fc_w.astype(np.float64).T + fc_b

    exp = np_ref()
    got, res = run(x, w_ih0, w_ih_rest, w_hh, b_ih, b_hh, fc_w, fc_b, T=T)
    err = np.abs(got - exp)
    rel = np.linalg.norm(got - exp) / np.linalg.norm(exp)
    print("max abs err:", err.max(), "rel:", rel)
    print("exec_time_ns:", res.exec_time_ns)


# revision 14
# speedup vs baseline: 3.2005x; 1.0022x over previous
"""GRU (5-layer, H=128) Trainium2 Bass kernel.

Strategy: pure data parallel over batch (64 / 8 cores = 8 per core).
Per core, the 5 layers run as a chunk-staggered wavefront (chunk C=16
timesteps): layer l processes chunk (m - l) during "round" m.  All
per-step elementwise work for the 5 active layers is batched into
wide ops on [128, nl*8] tiles.

Perf structure:
  - The per-step bottleneck is the tensor engine's 15 LDWEIGHTS
    (5 layers x 3 gates) per recurrence step.  The hidden-to-hidden
    weights are stored fp8 (e3m4) so FWL streams them 4B/cycle
    (~27ns vs ~53ns bf16); the moving operand (h) stays bf16
    (mixed-dtype matmul).  All gate weights/biases are pre-scaled by
    2^7 so fp8 hits its normal range; the scale is unwound for free
    via the activation `scale` operand on sigmoid/tanh.
  - Input-to-hidden gates are precomputed per chunk directly into
    PSUM (bias via rank<=4 ones-trick matmuls, then ih matmuls);
    ih weights stay bf16 (x128) -- accuracy, and their LDWEIGHTS
    hide under the larger N=128 matmuls.
  - x is pre-transposed on the host ([IN,T,B] layout), removing the
    per-chunk PE transpose + copy of the old design.
  - Elementwise chain per step (batched over active layers),
    critical path 6 ops:
      sig -> r*hn -> +xn -> tanh -> (z-1)*n [stt] -> p - (z-1)*n
    with p = z*h computed off-path on GPSIMD.

Layouts (per core, SBUF):
  hW  [128, 5, 577, 8]  h history; slot W stores h_l(t) at W = t + 16*l + 1
                        (W = 16l holds the per-layer zero initial state)
  xc  [128, C, 4, 8]    current input chunk (DMA'd from host-transposed x)
  P_all (PSUM) [128, 3, 5, 16, 8]  r/z/xn pre-activations for one chunk
  P_hn  (PSUM) [128, 5, 16, 8]     W_hn h + b_hhn for one chunk
"""

import os
import sys

for p in ("/opt/trn_rl_repo", "/opt/pypackages"):
    if p not in sys.path:
        sys.path.append(p)

import numpy as np
import ml_dtypes

# timing-ablation probes (GRU_PROBE env): "nochain" drops the elementwise
# chain (PE free-runs -> PE throughput floor); "nope" drops the per-step
# hh matmuls instead. Output is numerically invalid; timing-only.
PROBE = os.environ.get("GRU_PROBE", "")

BFNP = ml_dtypes.bfloat16
F8NP = ml_dtypes.float8_e3m4

import concourse.bass as bass  # noqa: F401
import concourse.mybir as mybir
import concourse.tile as tile
from concourse import bacc

F32 = mybir.dt.float32
BF16 = mybir.dt.bfloat16
FP8 = mybir.dt.float8e3
AF = mybir.ActivationFunctionType
ALU = mybir.AluOpType

H = 128
L = 5
NCORE = 8
BC = 8  # batch per core
IN = 512
OUT = 96
C = 16  # chunk (timesteps)

WSCALE = 128.0  # gate weights/biases pre-scaled by this; unwound in ACT
INV_WSCALE = 1.0 / WSCALE


def build_nc(T=512):
    NCH = T // C
    NR = NCH + L - 1
    WDIM = T + C * (L - 1) + 1

    nc = bacc.Bacc("TRN2", target_bir_lowering=False, debug=False)

    xT = nc.dram_tensor("xT", [H, T, 4, BC], BF16, kind="ExternalInput")
    whhT = nc.dram_tensor("whhT", [H, L, 3, H], FP8, kind="ExternalInput")
    wihT = nc.dram_tensor("wihT", [H, L - 1, 3, H], BF16, kind="ExternalInput")
    wih0T = nc.dram_tensor("wih0T", [H, 4, 3, H], BF16, kind="ExternalInput")
    fcT = nc.dram_tensor("fcT", [H, OUT], BF16, kind="ExternalInput")
    biasP = nc.dram_tensor("biasP", [4, 4, H], BF16, kind="ExternalInput")
    oh4 = nc.dram_tensor("oh4", [4, 512], BF16, kind="ExternalInput")
    bhn = nc.dram_tensor("bhn", [4, H], BF16, kind="ExternalInput")
    bl4 = nc.dram_tensor("bl4", [1, H], BF16, kind="ExternalInput")
    on1 = nc.dram_tensor("on1", [1, H], BF16, kind="ExternalInput")
    fcb = nc.dram_tensor("fcb", [BC, OUT], F32, kind="ExternalInput")
    y = nc.dram_tensor("y", [BC, OUT], F32, kind="ExternalOutput")

    with tile.TileContext(nc) as tc:
        with (
            tc.tile_pool(name="persist", bufs=1) as pp,
            tc.tile_pool(name="xsrc", bufs=3) as xsp,
            tc.tile_pool(name="tmp", bufs=3) as tp,
            tc.tile_pool(name="pall", bufs=1, space="PSUM") as pallp,
            tc.tile_pool(name="phn", bufs=1, space="PSUM") as phnp,
            tc.tile_pool(name="pfc", bufs=1, space="PSUM") as pfcp,
        ):
            hW = pp.tile([H, L, WDIM, BC], BF16, tag="hW")
            whh_sb = pp.tile([H, L, 3, H], FP8, tag="whh")
            wih_sb = pp.tile([H, L - 1, 3, H], BF16, tag="wih")
            wih0_sb = pp.tile([H, 4, 3, H], BF16, tag="wih0")
            fcT_sb = pp.tile([H, OUT], BF16, tag="fcT")
            biasP_sb = pp.tile([4, 4, H], BF16, tag="biasP")
            oh4_sb = pp.tile([4, 512], BF16, tag="oh4")
            bhn_sb = pp.tile([4, H], BF16, tag="bhn")
            bl4_sb = pp.tile([1, H], BF16, tag="bl4")
            on1_sb = pp.tile([1, H], BF16, tag="on1")
            fcb_sb = pp.tile([BC, OUT], F32, tag="fcb")

            nc.sync.dma_start(whh_sb[:, :, :, :], whhT[:, :, :, :])
            nc.sync.dma_start(wih0_sb[:, :, :, :], wih0T[:, :, :, :])
            nc.sync.dma_start(wih_sb[:, :, :, :], wihT[:, :, :, :])
            nc.sync.dma_start(biasP_sb[:, :, :], biasP[:, :, :])
            nc.sync.dma_start(oh4_sb[:, :], oh4[:, :])
            nc.sync.dma_start(bhn_sb[:, :], bhn[:, :])
            nc.sync.dma_start(bl4_sb[:, :], bl4[:, :])
            nc.sync.dma_start(on1_sb[:, :], on1[:, :])
            nc.sync.dma_start(fcT_sb[:, :], fcT[:, :])
            nc.sync.dma_start(fcb_sb[:, :], fcb[:, :])

            for l in range(L):
                nc.vector.memset(hW[:, l, C * l, :], 0.0)

            def load_x_chunk(m):
                xc = xsp.tile([H, C, 4, BC], BF16, tag="xc")
                nc.sync.dma_start(xc[:, :, :, :], xT[:, m * C : (m + 1) * C, :, :])
                return xc

            xc_cur = load_x_chunk(0)

            for m in range(NR):
                la0 = max(0, m - (NCH - 1))
                la1 = min(L - 1, m)
                sl = slice(la0, la1 + 1)

                P_all = pallp.tile([H, 3, L, C, BC], F32, tag="P_all")
                P_hn = phnp.tile([H, L, C, BC], F32, tag="P_hn")
                Pfl = P_all[:, :, :, :, :].rearrange("p g l c b -> p (g l c b)")
                Phfl = P_hn[:, :, :, :].rearrange("p l c b -> p (l c b)")

                # bias accumulation (start=True) via rank<=4 ones-trick
                for bk in range(4):
                    kk = 4 if bk < 3 else 3
                    N = 512 if bk < 3 else 384
                    nc.tensor.matmul(
                        Pfl[:, bk * 512 : bk * 512 + N],
                        biasP_sb[0:kk, bk, :],
                        oh4_sb[0:kk, 0:N],
                        start=True,
                        stop=False,
                        skip_group_check=True,
                    )
                nc.tensor.matmul(
                    Phfl[:, 0:512],
                    bhn_sb[0:4, :],
                    oh4_sb[0:4, 0:512],
                    start=True,
                    stop=False,
                    skip_group_check=True,
                )
                nc.tensor.matmul(
                    Phfl[:, 512:640],
                    bl4_sb[0:1, :],
                    on1_sb[0:1, 0:H],
                    start=True,
                    stop=False,
                    skip_group_check=True,
                )

                # input-gate (ih) chunk matmuls
                if m < NCH:  # layer 0 reads x chunk m
                    for g in range(3):
                        for ki in range(4):
                            nc.tensor.matmul(
                                P_all[:, g, 0, :, :],
                                wih0_sb[:, ki, g, :],
                                xc_cur[:, :, ki, :],
                                start=False,
                                stop=False,
                                skip_group_check=True,
                            )
                for l in range(max(1, la0), la1 + 1):
                    for g in range(3):
                        nc.tensor.matmul(
                            P_all[:, g, l, :, :],
                            wih_sb[:, l - 1, g, :],
                            hW[:, l - 1, C * m - 15 : C * m + 1, :],
                            start=False,
                            stop=False,
                            skip_group_check=True,
                        )

                if m + 1 < NCH:
                    xc_cur = load_x_chunk(m + 1)

                for j in range(C):
                    base = C * m + j
                    # hidden-gate matmuls for this step, in r -> hn -> z
                    # order: the critical path needs only r and hn first
                    # (z is consumed ~1us later by stt/pt)
                    if PROBE != "nope":
                        for g in (0, 2, 1):
                            for l in range(la0, la1 + 1):
                                dest = (
                                    P_all[:, g, l, j, :]
                                    if g < 2
                                    else P_hn[:, l, j, :]
                                )
                                nc.tensor.matmul(
                                    dest,
                                    whh_sb[:, l, g, :],
                                    hW[:, l, base, :],
                                    start=False,
                                    stop=True,
                                    skip_group_check=True,
                                )
                    if PROBE == "nochain":
                        continue
                    # elementwise chain, batched over active layers.
                    # r-sigmoid is split from z-sigmoid: it is half-size and
                    # starts after only the r matmuls; z-sigmoid is off the
                    # critical path (z is consumed ~1us later by stt/pt).
                    przt = tp.tile([H, L, BC], F32, tag="pr")
                    rzs = tp.tile([H, L, BC], BF16, tag="rzs")
                    rnt = tp.tile([H, L, BC], F32, tag="rn")
                    npret = tp.tile([H, L, BC], F32, tag="npre")
                    nt = tp.tile([H, L, BC], BF16, tag="nt")
                    tneg = tp.tile([H, L, BC], BF16, tag="tneg")
                    pt = tp.tile([H, L, BC], BF16, tag="pt")
                    nc.scalar.activation(
                        przt[:, sl, :], P_all[:, 0, sl, j, :], AF.Sigmoid,
                        scale=INV_WSCALE,
                    )
                    nc.scalar.activation(
                        rzs[:, sl, :], P_all[:, 1, sl, j, :], AF.Sigmoid,
                        scale=INV_WSCALE,
                    )
                    # p = z*h (off critical path)
                    nc.gpsimd.tensor_tensor(
                        pt[:, sl, :], rzs[:, sl, :], hW[:, sl, base, :], ALU.mult
                    )
                    nc.vector.tensor_tensor(
                        rnt[:, sl, :], przt[:, sl, :], P_hn[:, sl, j, :], ALU.mult
                    )
                    nc.vector.tensor_tensor(
                        npret[:, sl, :], rnt[:, sl, :], P_all[:, 2, sl, j, :], ALU.add
                    )
                    nc.scalar.activation(
                        nt[:, sl, :], npret[:, sl, :], AF.Tanh, scale=INV_WSCALE
                    )
                    # tneg = (z - 1) * n
                    nc.vector.scalar_tensor_tensor(
                        tneg[:, sl, :], rzs[:, sl, :], 1.0, nt[:, sl, :],
                        ALU.subtract, ALU.mult,
                    )
                    # h' = p - tneg = z*h + (1-z)*n
                    nc.vector.tensor_tensor(
                        hW[:, sl, base + 1, :], pt[:, sl, :], tneg[:, sl, :],
                        ALU.subtract,
                    )

            # final FC on last timestep of layer 4
            pfc = pfcp.tile([BC, OUT], F32, tag="fc")
            nc.tensor.matmul(
                pfc[:, :],
                hW[:, L - 1, WDIM - 1, :],
                fcT_sb[:, :],
                start=True,
                stop=True,
                skip_group_check=True,
            )
            out_sb = pp.tile([BC, OUT], F32, tag="out")
            nc.vector.tensor_tensor(out_sb[:, :], pfc[:, :], fcb_sb[:, :], ALU.add)
            nc.sync.dma_start(y[:, :], out_sb[:, :])

    nc.compile()
    return nc


def prep_shared(w_ih0, w_ih_rest, w_hh, b_ih, b_hh, fc_w, fc_b):
    d = {}
    whhT = np.empty([H, L, 3, H], np.float32)
    for l in range(L):
        for g in range(3):
            whhT[:, l, g, :] = w_hh[l, g * H : (g + 1) * H, :].T
    d["whhT"] = (whhT * WSCALE).astype(F8NP)
    wihT = np.empty([H, L - 1, 3, H], np.float32)
    for l in range(1, L):
        for g in range(3):
            wihT[:, l - 1, g, :] = w_ih_rest[l - 1, g * H : (g + 1) * H, :].T
    d["wihT"] = (wihT * WSCALE).astype(BFNP)
    wih0T = np.empty([H, 4, 3, H], np.float32)
    for ki in range(4):
        for g in range(3):
            wih0T[:, ki, g, :] = w_ih0[g * H : (g + 1) * H, ki * H : (ki + 1) * H].T
    d["wih0T"] = (wih0T * WSCALE).astype(BFNP)
    d["fcT"] = np.ascontiguousarray(fc_w.T).astype(BFNP)
    biasP = np.zeros([4, 4, H], np.float32)
    for i in range(15):  # chunk index i = g*5 + l
        g, l = divmod(i, 5)
        b = b_ih[l, g * H : (g + 1) * H].astype(np.float32)
        if g < 2:
            b = b + b_hh[l, g * H : (g + 1) * H]
        biasP[i % 4, i // 4, :] = b  # dims: [k, bank, H]
    d["biasP"] = (biasP * WSCALE).astype(BFNP)
    oh4 = np.zeros([4, 512], np.float32)
    for k in range(4):
        oh4[k, k * H : (k + 1) * H] = 1.0
    d["oh4"] = oh4.astype(BFNP)
    d["bhn"] = (b_hh[0:4, 2 * H : 3 * H].astype(np.float32) * WSCALE).astype(BFNP)
    d["bl4"] = (b_hh[4:5, 2 * H : 3 * H].astype(np.float32) * WSCALE).astype(BFNP)
    d["on1"] = np.ones([1, H], np.float32).astype(BFNP)
    d["fcb"] = np.tile(fc_b.astype(np.float32)[None, :], (BC, 1))
    return d


_NC_CACHE = {}


def run(x, w_ih0, w_ih_rest, w_hh, b_ih, b_hh, fc_w, fc_b, T=512, **run_kwargs):
    from concourse.bass_utils import run_bass_kernel_spmd

    if T not in _NC_CACHE:
        _NC_CACHE[T] = build_nc(T)
    nc = _NC_CACHE[T]
    shared = prep_shared(
        np.asarray(w_ih0), np.asarray(w_ih_rest), np.asarray(w_hh),
        np.asarray(b_ih), np.asarray(b_hh), np.asarray(fc_w), np.asarray(fc_b),
    )
    x = np.asarray(x)
    in_maps = []
    for c in range(NCORE):
        m = dict(shared)
        m["xT"] = prep_x(x, c, T)
        in_maps.append(m)
    res = run_bass_kernel_spmd(nc, in_maps, core_ids=list(range(NCORE)), **run_kwargs)
    out = np.concatenate([res.results[c]["y"] for c in range(NCORE)], axis=0)
    return out, res


def prep_x(x, c, T):
    xs = x[c * BC : (c + 1) * BC, :T, :]  # [BC, T, IN]
    # -> [H, T, 4, BC]: element [p, t, ki, b] = x[b, t, ki*128+p]
    xt = xs.transpose(2, 1, 0).reshape(4, H, T, BC)  # [ki, p, t, b]
    return np.ascontiguousarray(xt.transpose(1, 2, 0, 3)).astype(BFNP)


def kernel(x, w_ih0, w_ih_rest, w_hh, b_ih, b_hh, fc_w, fc_b):
    out, _ = run(x, w_ih0, w_ih_rest, w_hh, b_ih, b_hh, fc_w, fc_b, T=512)
    return out.astype(np.float32)


if __name__ == "__main__":
    # quick smoke test at small T against a numpy reference
    T = int(sys.argv[1]) if len(sys.argv) > 1 else 64
    rng = np.random.default_rng(0)
    s = 1.0 / np.sqrt(H)
    u = lambda *sh: rng.uniform(-s, s, sh).astype(np.float32)
    x = rng.standard_normal((64, T, IN), dtype=np.float32)
    w_ih0 = u(3 * H, IN)
    w_ih_rest = u(L - 1, 3 * H, H)
    w_hh = u(L, 3 * H, H)
    b_ih = u(L, 3 * H)
    b_hh = u(L, 3 * H)
    fc_w = u(OUT, H)
    fc_b = u(OUT)

    def np_ref():
        sig = lambda v: 1.0 / (1.0 + np.exp(-v))
        h_in = x.astype(np.float64)
        for l in range(L):
            wi = (w_ih0 if l == 0 else w_ih_rest[l - 1]).astype(np.float64)
            wh = w_hh[l].astype(np.float64)
            gx = np.einsum("bti,gi->btg", h_in, wi) + b_ih[l]
            h = np.zeros((64, H))
            hs = []
            for t in range(T):
                gh = h @ wh.T + b_hh[l]
                xr, xz, xn = np.split(gx[:, t], 3, -1)
                hr, hz, hn = np.split(gh, 3, -1)
                r = sig(xr + hr)
                z = sig(xz + hz)
                n = np.tanh(xn + r * hn)
                h = (1 - z) * n + z * h
                hs.append(h)
            h_in = np.stack(hs, 1)
        return h_in[:, -1] @ fc_w.astype(np.float64).T + fc_b

    exp = np_ref()
    got, res = run(x, w_ih0, w_ih_rest, w_hh, b_ih, b_hh, fc_w, fc_b, T=T)
    err = np.abs(got - exp)
    rel = np.linalg.norm(got - exp) / np.linalg.norm(exp)
    print("max abs err:", err.max(), "rel:", rel)
    print("exec_time_ns:", res.exec_time_ns)


# revision 15
# speedup vs baseline: 3.2493x; 1.0153x over previous
"""GRU (5-layer, H=128) Trainium2 Bass kernel.

Strategy: pure data parallel over batch (64 / 8 cores = 8 per core).
Per core, the 5 layers run as a chunk-staggered wavefront (chunk C=16
timesteps): layer l processes chunk (m - l) during "round" m.  All
per-step elementwise work for the 5 active layers is batched into
wide ops on [128, nl*8] tiles.

Perf structure:
  - The per-step bottleneck is the tensor engine's 15 LDWEIGHTS
    (5 layers x 3 gates) per recurrence step.  The hidden-to-hidden
    weights are stored fp8 (e3m4) so FWL streams them 4B/cycle
    (~27ns vs ~53ns bf16); the moving operand (h) stays bf16
    (mixed-dtype matmul).  All gate weights/biases are pre-scaled by
    2^7 so fp8 hits its normal range; the scale is unwound for free
    via the activation `scale` operand on sigmoid/tanh.
  - Input-to-hidden gates are precomputed per chunk directly into
    PSUM (bias via rank<=4 ones-trick matmuls, then ih matmuls);
    ih weights stay bf16 (x128) -- accuracy, and their LDWEIGHTS
    hide under the larger N=128 matmuls.
  - x is pre-transposed on the host ([IN,T,B] layout), removing the
    per-chunk PE transpose + copy of the old design.
  - Elementwise chain per step (batched over active layers),
    critical path 6 ops:
      sig -> r*hn -> +xn -> tanh -> (z-1)*n [stt] -> p - (z-1)*n
    with p = z*h computed off-path on GPSIMD.

Layouts (per core, SBUF):
  hW  [128, 5, 577, 8]  h history; slot W stores h_l(t) at W = t + 16*l + 1
                        (W = 16l holds the per-layer zero initial state)
  xc  [128, C, 4, 8]    current input chunk (DMA'd from host-transposed x)
  P_all (PSUM) [128, 3, 5, 16, 8]  r/z/xn pre-activations for one chunk
  P_hn  (PSUM) [128, 5, 16, 8]     W_hn h + b_hhn for one chunk
"""

import os
import sys

for p in ("/opt/trn_rl_repo", "/opt/pypackages"):
    if p not in sys.path:
        sys.path.append(p)

import numpy as np
import ml_dtypes

# timing-ablation probes (GRU_PROBE env): "nochain" drops the elementwise
# chain (PE free-runs -> PE throughput floor); "nope" drops the per-step
# hh matmuls instead. Output is numerically invalid; timing-only.
PROBE = os.environ.get("GRU_PROBE", "")

BFNP = ml_dtypes.bfloat16
F8NP = ml_dtypes.float8_e3m4

import concourse.bass as bass  # noqa: F401
import concourse.mybir as mybir
import concourse.tile as tile
from concourse import bacc

F32 = mybir.dt.float32
BF16 = mybir.dt.bfloat16
FP8 = mybir.dt.float8e3
AF = mybir.ActivationFunctionType
ALU = mybir.AluOpType

H = 128
L = 5
NCORE = 8
BC = 8  # batch per core
IN = 512
OUT = 96
C = 8  # chunk (timesteps)

WSCALE = 128.0  # gate weights/biases pre-scaled by this; unwound in ACT
INV_WSCALE = 1.0 / WSCALE


def build_nc(T=512):
    NCH = T // C
    NR = NCH + L - 1
    WDIM = T + C * (L - 1) + 1

    nc = bacc.Bacc("TRN2", target_bir_lowering=False, debug=False)

    xT = nc.dram_tensor("xT", [H, T, 4, BC], BF16, kind="ExternalInput")
    whhT = nc.dram_tensor("whhT", [H, L, 3, H], FP8, kind="ExternalInput")
    wihT = nc.dram_tensor("wihT", [H, L - 1, 3, H], BF16, kind="ExternalInput")
    wih0T = nc.dram_tensor("wih0T", [H, 4, 3, H], BF16, kind="ExternalInput")
    fcT = nc.dram_tensor("fcT", [H, OUT], BF16, kind="ExternalInput")
    biasP = nc.dram_tensor("biasP", [8, 2, H], BF16, kind="ExternalInput")
    oh8 = nc.dram_tensor("oh8", [8, 512], BF16, kind="ExternalInput")
    bhn5 = nc.dram_tensor("bhn5", [5, H], BF16, kind="ExternalInput")
    fcb = nc.dram_tensor("fcb", [BC, OUT], F32, kind="ExternalInput")
    y = nc.dram_tensor("y", [BC, OUT], F32, kind="ExternalOutput")

    with tile.TileContext(nc) as tc:
        with (
            tc.tile_pool(name="persist", bufs=1) as pp,
            tc.tile_pool(name="xsrc", bufs=3) as xsp,
            tc.tile_pool(name="tmp", bufs=3) as tp,
            tc.tile_pool(name="pall", bufs=2, space="PSUM") as pallp,
            tc.tile_pool(name="phn", bufs=2, space="PSUM") as phnp,
            tc.tile_pool(name="pfc", bufs=1, space="PSUM") as pfcp,
        ):
            hW = pp.tile([H, L, WDIM, BC], BF16, tag="hW")
            whh_sb = pp.tile([H, L, 3, H], FP8, tag="whh")
            wih_sb = pp.tile([H, L - 1, 3, H], BF16, tag="wih")
            wih0_sb = pp.tile([H, 4, 3, H], BF16, tag="wih0")
            fcT_sb = pp.tile([H, OUT], BF16, tag="fcT")
            biasP_sb = pp.tile([8, 2, H], BF16, tag="biasP")
            oh8_sb = pp.tile([8, 512], BF16, tag="oh8")
            bhn5_sb = pp.tile([5, H], BF16, tag="bhn5")
            fcb_sb = pp.tile([BC, OUT], F32, tag="fcb")

            nc.sync.dma_start(whh_sb[:, :, :, :], whhT[:, :, :, :])
            nc.sync.dma_start(wih0_sb[:, :, :, :], wih0T[:, :, :, :])
            nc.sync.dma_start(wih_sb[:, :, :, :], wihT[:, :, :, :])
            nc.sync.dma_start(biasP_sb[:, :, :], biasP[:, :, :])
            nc.sync.dma_start(oh8_sb[:, :], oh8[:, :])
            nc.sync.dma_start(bhn5_sb[:, :], bhn5[:, :])
            nc.sync.dma_start(fcT_sb[:, :], fcT[:, :])
            nc.sync.dma_start(fcb_sb[:, :], fcb[:, :])

            for l in range(L):
                nc.vector.memset(hW[:, l, C * l, :], 0.0)

            def load_x_chunk(m):
                xc = xsp.tile([H, C, 4, BC], BF16, tag="xc")
                nc.sync.dma_start(xc[:, :, :, :], xT[:, m * C : (m + 1) * C, :, :])
                return xc

            xc_cur = load_x_chunk(0)

            for m in range(NR):
                la0 = max(0, m - (NCH - 1))
                la1 = min(L - 1, m)
                sl = slice(la0, la1 + 1)

                P_all = pallp.tile([H, 3, L, C, BC], F32, tag="P_all")
                P_hn = phnp.tile([H, L, C, BC], F32, tag="P_hn")
                Pfl = P_all[:, :, :, :, :].rearrange("p g l c b -> p (g l c b)")
                Phfl = P_hn[:, :, :, :].rearrange("p l c b -> p (l c b)")

                # bias accumulation (start=True) via rank<=8 ones-trick
                for bk in range(2):
                    kk = 8 if bk == 0 else 7
                    N = 512 if bk == 0 else 448
                    nc.tensor.matmul(
                        Pfl[:, bk * 512 : bk * 512 + N],
                        biasP_sb[0:kk, bk, :],
                        oh8_sb[0:kk, 0:N],
                        start=True,
                        stop=False,
                        skip_group_check=True,
                    )
                nc.tensor.matmul(
                    Phfl[:, 0 : 5 * C * BC],
                    bhn5_sb[0:5, :],
                    oh8_sb[0:5, 0 : 5 * C * BC],
                    start=True,
                    stop=False,
                    skip_group_check=True,
                )

                # input-gate (ih) chunk matmuls
                if m < NCH:  # layer 0 reads x chunk m
                    for g in range(3):
                        for ki in range(4):
                            nc.tensor.matmul(
                                P_all[:, g, 0, :, :],
                                wih0_sb[:, ki, g, :],
                                xc_cur[:, :, ki, :],
                                start=False,
                                stop=False,
                                skip_group_check=True,
                            )
                for l in range(max(1, la0), la1 + 1):
                    for g in range(3):
                        nc.tensor.matmul(
                            P_all[:, g, l, :, :],
                            wih_sb[:, l - 1, g, :],
                            hW[:, l - 1, C * m - (C - 1) : C * m + 1, :],
                            start=False,
                            stop=False,
                            skip_group_check=True,
                        )

                if m + 1 < NCH:
                    xc_cur = load_x_chunk(m + 1)

                for j in range(C):
                    base = C * m + j
                    # hidden-gate matmuls for this step, in r -> hn -> z
                    # order: the critical path needs only r and hn first
                    # (z is consumed ~1us later by stt/pt)
                    if PROBE != "nope":
                        for g in (0, 2, 1):
                            for l in range(la0, la1 + 1):
                                dest = (
                                    P_all[:, g, l, j, :]
                                    if g < 2
                                    else P_hn[:, l, j, :]
                                )
                                nc.tensor.matmul(
                                    dest,
                                    whh_sb[:, l, g, :],
                                    hW[:, l, base, :],
                                    start=False,
                                    stop=True,
                                    skip_group_check=True,
                                )
                    if PROBE == "nochain":
                        continue
                    # elementwise chain, batched over active layers.
                    # r-sigmoid is split from z-sigmoid: it is half-size and
                    # starts after only the r matmuls; z-sigmoid is off the
                    # critical path (z is consumed ~1us later by stt/pt).
                    przt = tp.tile([H, L, BC], F32, tag="pr")
                    rzs = tp.tile([H, L, BC], BF16, tag="rzs")
                    rnt = tp.tile([H, L, BC], F32, tag="rn")
                    npret = tp.tile([H, L, BC], F32, tag="npre")
                    nt = tp.tile([H, L, BC], BF16, tag="nt")
                    tneg = tp.tile([H, L, BC], BF16, tag="tneg")
                    pt = tp.tile([H, L, BC], BF16, tag="pt")
                    nc.scalar.activation(
                        przt[:, sl, :], P_all[:, 0, sl, j, :], AF.Sigmoid,
                        scale=INV_WSCALE,
                    )
                    nc.scalar.activation(
                        rzs[:, sl, :], P_all[:, 1, sl, j, :], AF.Sigmoid,
                        scale=INV_WSCALE,
                    )
                    # p = z*h (off critical path)
                    nc.gpsimd.tensor_tensor(
                        pt[:, sl, :], rzs[:, sl, :], hW[:, sl, base, :], ALU.mult
                    )
                    nc.vector.tensor_tensor(
                        rnt[:, sl, :], przt[:, sl, :], P_hn[:, sl, j, :], ALU.mult
                    )
                    nc.vector.tensor_tensor(
                        npret[:, sl, :], rnt[:, sl, :], P_all[:, 2, sl, j, :], ALU.add
                    )
                    nc.scalar.activation(
                        nt[:, sl, :], npret[:, sl, :], AF.Tanh, scale=INV_WSCALE
                    )
                    # tneg = (z - 1) * n
                    nc.vector.scalar_tensor_tensor(
                        tneg[:, sl, :], rzs[:, sl, :], 1.0, nt[:, sl, :],
                        ALU.subtract, ALU.mult,
                    )
                    # h' = p - tneg = z*h + (1-z)*n
                    nc.vector.tensor_tensor(
                        hW[:, sl, base + 1, :], pt[:, sl, :], tneg[:, sl, :],
                        ALU.subtract,
                    )

            # final FC on last timestep of layer 4
            pfc = pfcp.tile([BC, OUT], F32, tag="fc")
            nc.tensor.matmul(
                pfc[:, :],
                hW[:, L - 1, WDIM - 1, :],
                fcT_sb[:, :],
                start=True,
                stop=True,
                skip_group_check=True,
            )
            out_sb = pp.tile([BC, OUT], F32, tag="out")
            nc.vector.tensor_tensor(out_sb[:, :], pfc[:, :], fcb_sb[:, :], ALU.add)
            nc.sync.dma_start(y[:, :], out_sb[:, :])

    nc.compile()
    return nc


def prep_shared(w_ih0, w_ih_rest, w_hh, b_ih, b_hh, fc_w, fc_b):
    d = {}
    whhT = np.empty([H, L, 3, H], np.float32)
    for l in range(L):
        for g in range(3):
            whhT[:, l, g, :] = w_hh[l, g * H : (g + 1) * H, :].T
    d["whhT"] = (whhT * WSCALE).astype(F8NP)
    wihT = np.empty([H, L - 1, 3, H], np.float32)
    for l in range(1, L):
        for g in range(3):
            wihT[:, l - 1, g, :] = w_ih_rest[l - 1, g * H : (g + 1) * H, :].T
    d["wihT"] = (wihT * WSCALE).astype(BFNP)
    wih0T = np.empty([H, 4, 3, H], np.float32)
    for ki in range(4):
        for g in range(3):
            wih0T[:, ki, g, :] = w_ih0[g * H : (g + 1) * H, ki * H : (ki + 1) * H].T
    d["wih0T"] = (wih0T * WSCALE).astype(BFNP)
    d["fcT"] = np.ascontiguousarray(fc_w.T).astype(BFNP)
    CB = C * BC  # cols per (gate, layer) combo
    biasP = np.zeros([8, 2, H], np.float32)
    for i in range(15):  # combo index i = g*5 + l
        g, l = divmod(i, 5)
        b = b_ih[l, g * H : (g + 1) * H].astype(np.float32)
        if g < 2:
            b = b + b_hh[l, g * H : (g + 1) * H]
        biasP[i % 8, i // 8, :] = b  # dims: [k, bank, H]
    d["biasP"] = (biasP * WSCALE).astype(BFNP)
    oh8 = np.zeros([8, 512], np.float32)
    for k in range(8):
        oh8[k, k * CB : (k + 1) * CB] = 1.0
    d["oh8"] = oh8.astype(BFNP)
    d["bhn5"] = (b_hh[:, 2 * H : 3 * H].astype(np.float32) * WSCALE).astype(BFNP)
    d["fcb"] = np.tile(fc_b.astype(np.float32)[None, :], (BC, 1))
    return d


_NC_CACHE = {}


def run(x, w_ih0, w_ih_rest, w_hh, b_ih, b_hh, fc_w, fc_b, T=512, **run_kwargs):
    from concourse.bass_utils import run_bass_kernel_spmd

    if T not in _NC_CACHE:
        _NC_CACHE[T] = build_nc(T)
    nc = _NC_CACHE[T]
    shared = prep_shared(
        np.asarray(w_ih0), np.asarray(w_ih_rest), np.asarray(w_hh),
        np.asarray(b_ih), np.asarray(b_hh), np.asarray(fc_w), np.asarray(fc_b),
    )
    x = np.asarray(x)
    in_maps = []
    for c in range(NCORE):
        m = dict(shared)
        m["xT"] = prep_x(x, c, T)
        in_maps.append(m)
    res = run_bass_kernel_spmd(nc, in_maps, core_ids=list(range(NCORE)), **run_kwargs)
    out = np.concatenate([res.results[c]["y"] for c in range(NCORE)], axis=0)
    return out, res


def prep_x(x, c, T):
    xs = x[c * BC : (c + 1) * BC, :T, :]  # [BC, T, IN]
    # -> [H, T, 4, BC]: element [p, t, ki, b] = x[b, t, ki*128+p]
    xt = xs.transpose(2, 1, 0).reshape(4, H, T, BC)  # [ki, p, t, b]
    return np.ascontiguousarray(xt.transpose(1, 2, 0, 3)).astype(BFNP)


def kernel(x, w_ih0, w_ih_rest, w_hh, b_ih, b_hh, fc_w, fc_b):
    out, _ = run(x, w_ih0, w_ih_rest, w_hh, b_ih, b_hh, fc_w, fc_b, T=512)
    return out.astype(np.float32)


if __name__ == "__main__":
    # quick smoke test at small T against a numpy reference
    T = int(sys.argv[1]) if len(sys.argv) > 1 else 64
    rng = np.random.default_rng(0)
    s = 1.0 / np.sqrt(H)
    u = lambda *sh: rng.uniform(-s, s, sh).astype(np.float32)
    x = rng.standard_normal((64, T, IN), dtype=np.float32)
    w_ih0 = u(3 * H, IN)
    w_ih_rest = u(L - 1, 3 * H, H)
    w_hh = u(L, 3 * H, H)
    b_ih = u(L, 3 * H)
    b_hh = u(L, 3 * H)
    fc_w = u(OUT, H)
    fc_b = u(OUT)

    def np_ref():
        sig = lambda v: 1.0 / (1.0 + np.exp(-v))
        h_in = x.astype(np.float64)
        for l in range(L):
            wi = (w_ih0 if l == 0 else w_ih_rest[l - 1]).astype(np.float64)
            wh = w_hh[l].astype(np.float64)
            gx = np.einsum("bti,gi->btg", h_in, wi) + b_ih[l]
            h = np.zeros((64, H))
            hs = []
            for t in range(T):
                gh = h @ wh.T + b_hh[l]
                xr, xz, xn = np.split(gx[:, t], 3, -1)
                hr, hz, hn = np.split(gh, 3, -1)
                r = sig(xr + hr)
                z = sig(xz + hz)
                n = np.tanh(xn + r * hn)
                h = (1 - z) * n + z * h
                hs.append(h)
            h_in = np.stack(hs, 1)
        return h_in[:, -1] @ fc_w.astype(np.float64).T + fc_b

    exp = np_ref()
    got, res = run(x, w_ih0, w_ih_rest, w_hh, b_ih, b_hh, fc_w, fc_b, T=T)
    err = np.abs(got - exp)
    rel = np.linalg.norm(got - exp) / np.linalg.norm(exp)
    print("max abs err:", err.max(), "rel:", rel)
    print("exec_time_ns:", res.exec_time_ns)


# revision 17
# speedup vs baseline: 3.4616x; 1.0653x over previous
"""GRU (5-layer, H=128) Trainium2 Bass kernel.

Strategy: pure data parallel over batch (64 / 8 cores = 8 per core).
Per core, the 5 layers run as a chunk-staggered wavefront (chunk C=8
timesteps): layer l processes chunk (m - l) during "round" m.  All
per-step elementwise work for the 5 active layers is batched into
wide ops on [128, nl*8] tiles.

Perf structure:
  - The per-step bottleneck is the tensor engine's 15 LDWEIGHTS
    (5 layers x 3 gates) per recurrence step.  The hidden-to-hidden
    weights are stored fp8 (e3m4) so FWL streams them 4B/cycle
    (~27ns vs ~53ns bf16); the moving operand (h) stays bf16
    (mixed-dtype matmul).  All gate weights/biases are pre-scaled by
    2^7 so fp8 hits its normal range; the scale is unwound for free
    via the activation `scale` operand on sigmoid/tanh.
  - Input-to-hidden gates are precomputed per chunk directly into
    PSUM (bias via rank<=4 ones-trick matmuls, then ih matmuls);
    ih weights stay bf16 (x128) -- accuracy, and their LDWEIGHTS
    hide under the larger N=128 matmuls.
  - x is pre-transposed on the host ([IN,T,B] layout), removing the
    per-chunk PE transpose + copy of the old design.
  - Elementwise chain per step (batched over active layers),
    critical path 6 ops:
      sig -> r*hn -> +xn -> tanh -> (z-1)*n [stt] -> p - (z-1)*n
    with p = z*h computed off-path on GPSIMD.

Layouts (per core, SBUF):
  hW  [128, 5, T+33, 8] h history; slot W stores h_l(t) at W = t + C*l + 1
                        (W = C*l holds the per-layer zero initial state)
  xc  [128, C, 4, 8]    current input chunk (DMA'd from host-transposed x)
  P_all (PSUM) [128, 3, 5, C, 8]  r/z/xn pre-activations for one chunk
  P_hn  (PSUM) [128, 5, C, 8]     W_hn h + b_hhn for one chunk
  (both double-buffered: C=8 halves them so bufs=2 fits the 8 PSUM banks,
   letting the next round's bias matmuls run during this round's chain)
"""

import os
import sys

for p in ("/opt/trn_rl_repo", "/opt/pypackages"):
    if p not in sys.path:
        sys.path.append(p)

import numpy as np
import ml_dtypes

# timing-ablation probes (GRU_PROBE env): "nochain" drops the elementwise
# chain (PE free-runs -> PE throughput floor); "nope" drops the per-step
# hh matmuls instead. Output is numerically invalid; timing-only.
PROBE = os.environ.get("GRU_PROBE", "")

BFNP = ml_dtypes.bfloat16
F8NP = ml_dtypes.float8_e3m4

import concourse.bass as bass  # noqa: F401
import concourse.mybir as mybir
import concourse.tile as tile
from concourse import bacc

F32 = mybir.dt.float32
BF16 = mybir.dt.bfloat16
FP8 = mybir.dt.float8e3
AF = mybir.ActivationFunctionType
ALU = mybir.AluOpType

H = 128
L = 5
NCORE = 8
BC = 8  # batch per core
IN = 512
OUT = 96
C = 8  # chunk (timesteps)

WSCALE = 128.0  # gate weights/biases pre-scaled by this; unwound in ACT
INV_WSCALE = 1.0 / WSCALE


def build_nc(T=512):
    NCH = T // C
    NR = NCH + L - 1
    WDIM = T + C * (L - 1) + 1

    nc = bacc.Bacc("TRN2", target_bir_lowering=False, debug=False)

    xT = nc.dram_tensor("xT", [H, T, 4, BC], BF16, kind="ExternalInput")
    whhT = nc.dram_tensor("whhT", [H, L, 3, H], FP8, kind="ExternalInput")
    wihT = nc.dram_tensor("wihT", [H, L - 1, 3, H], BF16, kind="ExternalInput")
    wih0T = nc.dram_tensor("wih0T", [H, 4, 3, H], BF16, kind="ExternalInput")
    fcT = nc.dram_tensor("fcT", [H, OUT], BF16, kind="ExternalInput")
    biasP = nc.dram_tensor("biasP", [8, 2, H], BF16, kind="ExternalInput")
    oh8 = nc.dram_tensor("oh8", [8, 512], BF16, kind="ExternalInput")
    bhn5 = nc.dram_tensor("bhn5", [5, H], BF16, kind="ExternalInput")
    fcb = nc.dram_tensor("fcb", [BC, OUT], F32, kind="ExternalInput")
    y = nc.dram_tensor("y", [BC, OUT], F32, kind="ExternalOutput")

    with tile.TileContext(nc) as tc:
        with (
            tc.tile_pool(name="persist", bufs=1) as pp,
            tc.tile_pool(name="xsrc", bufs=3) as xsp,
            tc.tile_pool(name="tmp", bufs=3) as tp,
            tc.tile_pool(name="pall", bufs=2, space="PSUM") as pallp,
            tc.tile_pool(name="phn", bufs=2, space="PSUM") as phnp,
            tc.tile_pool(name="pfc", bufs=1, space="PSUM") as pfcp,
        ):
            hW = pp.tile([H, L, WDIM, BC], BF16, tag="hW")
            whh_sb = pp.tile([H, L, 3, H], FP8, tag="whh")
            wih_sb = pp.tile([H, L - 1, 3, H], BF16, tag="wih")
            wih0_sb = pp.tile([H, 4, 3, H], BF16, tag="wih0")
            fcT_sb = pp.tile([H, OUT], BF16, tag="fcT")
            biasP_sb = pp.tile([8, 2, H], BF16, tag="biasP")
            oh8_sb = pp.tile([8, 512], BF16, tag="oh8")
            bhn5_sb = pp.tile([5, H], BF16, tag="bhn5")
            fcb_sb = pp.tile([BC, OUT], F32, tag="fcb")

            nc.sync.dma_start(whh_sb[:, :, :, :], whhT[:, :, :, :])
            nc.sync.dma_start(wih0_sb[:, :, :, :], wih0T[:, :, :, :])
            nc.sync.dma_start(wih_sb[:, :, :, :], wihT[:, :, :, :])
            nc.sync.dma_start(biasP_sb[:, :, :], biasP[:, :, :])
            nc.sync.dma_start(oh8_sb[:, :], oh8[:, :])
            nc.sync.dma_start(bhn5_sb[:, :], bhn5[:, :])
            nc.sync.dma_start(fcT_sb[:, :], fcT[:, :])
            nc.sync.dma_start(fcb_sb[:, :], fcb[:, :])

            for l in range(L):
                nc.vector.memset(hW[:, l, C * l, :], 0.0)

            def load_x_chunk(m):
                xc = xsp.tile([H, C, 4, BC], BF16, tag="xc")
                nc.sync.dma_start(xc[:, :, :, :], xT[:, m * C : (m + 1) * C, :, :])
                return xc

            xc_cur = load_x_chunk(0)

            def alloc_round():
                P_all = pallp.tile([H, 3, L, C, BC], F32, tag="P_all")
                P_hn = phnp.tile([H, L, C, BC], F32, tag="P_hn")
                return P_all, P_hn

            def emit_bias(P_all, P_hn):
                Pfl = P_all[:, :, :, :, :].rearrange("p g l c b -> p (g l c b)")
                Phfl = P_hn[:, :, :, :].rearrange("p l c b -> p (l c b)")
                # bias accumulation (start=True) via rank<=8 ones-trick
                for bk in range(2):
                    kk = 8 if bk == 0 else 7
                    N = 512 if bk == 0 else 448
                    nc.tensor.matmul(
                        Pfl[:, bk * 512 : bk * 512 + N],
                        biasP_sb[0:kk, bk, :],
                        oh8_sb[0:kk, 0:N],
                        start=True,
                        stop=False,
                        skip_group_check=True,
                    )
                nc.tensor.matmul(
                    Phfl[:, 0 : 5 * C * BC],
                    bhn5_sb[0:5, :],
                    oh8_sb[0:5, 0 : 5 * C * BC],
                    start=True,
                    stop=False,
                    skip_group_check=True,
                )

            def emit_x_ih(P_all, xc):
                # layer-0 input gates: no h dependency (x chunk via DMA)
                for g in range(3):
                    for ki in range(4):
                        nc.tensor.matmul(
                            P_all[:, g, 0, :, :],
                            wih0_sb[:, ki, g, :],
                            xc[:, :, ki, :],
                            start=False,
                            stop=False,
                            skip_group_check=True,
                        )

            def emit_h_ih(P_all, mm, c0, c1):
                # layers>=1 input gates for round mm, chunk columns [c0,c1):
                # the window slice needs only the previous round's steps
                # c0..c1-1, so the first half can issue mid-round and hide
                # in the chain-latency windows (PSUM is double-buffered)
                a0 = max(0, mm - (NCH - 1))
                a1 = min(L - 1, mm)
                for l in range(max(1, a0), a1 + 1):
                    for g in range(3):
                        nc.tensor.matmul(
                            P_all[:, g, l, c0:c1, :],
                            wih_sb[:, l - 1, g, :],
                            hW[:, l - 1, C * mm - (C - 1) + c0 : C * mm - (C - 1) + c1, :],
                            start=False,
                            stop=False,
                            skip_group_check=True,
                        )

            P_all, P_hn = alloc_round()
            emit_bias(P_all, P_hn)
            emit_x_ih(P_all, xc_cur)

            for m in range(NR):
                la0 = max(0, m - (NCH - 1))
                la1 = min(L - 1, m)
                sl = slice(la0, la1 + 1)

                xc_nxt = load_x_chunk(m + 1) if m + 1 < NCH else None
                P_nxt = Ph_nxt = None

                for j in range(C):
                    base = C * m + j
                    if j == 4 and m + 1 < NR:
                        # next round's prefill: bias (no deps), layer-0 ih
                        # (x only), and the first half of the h-dependent ih
                        # (needs only this round's steps 0..3, done by now)
                        P_nxt, Ph_nxt = alloc_round()
                        emit_bias(P_nxt, Ph_nxt)
                        if m + 1 < NCH:
                            emit_x_ih(P_nxt, xc_nxt)
                        emit_h_ih(P_nxt, m + 1, 0, C // 2)
                    # hidden-gate matmuls for this step, in r -> hn -> z
                    # order: the critical path needs only r and hn first
                    # (z is consumed ~1us later by stt/pt)
                    if PROBE != "nope":
                        for g in (0, 2, 1):
                            for l in range(la0, la1 + 1):
                                dest = (
                                    P_all[:, g, l, j, :]
                                    if g < 2
                                    else P_hn[:, l, j, :]
                                )
                                nc.tensor.matmul(
                                    dest,
                                    whh_sb[:, l, g, :],
                                    hW[:, l, base, :],
                                    start=False,
                                    stop=True,
                                    skip_group_check=True,
                                )
                    if PROBE == "nochain":
                        continue
                    # elementwise chain, batched over active layers.
                    # r-sigmoid is split from z-sigmoid: it is half-size and
                    # starts after only the r matmuls; z-sigmoid is off the
                    # critical path (z is consumed ~1us later by stt/pt).
                    przt = tp.tile([H, L, BC], F32, tag="pr")
                    rzs = tp.tile([H, L, BC], BF16, tag="rzs")
                    rnt = tp.tile([H, L, BC], F32, tag="rn")
                    npret = tp.tile([H, L, BC], F32, tag="npre")
                    nt = tp.tile([H, L, BC], BF16, tag="nt")
                    tneg = tp.tile([H, L, BC], BF16, tag="tneg")
                    pt = tp.tile([H, L, BC], BF16, tag="pt")
                    nc.scalar.activation(
                        przt[:, sl, :], P_all[:, 0, sl, j, :], AF.Sigmoid,
                        scale=INV_WSCALE,
                    )
                    nc.scalar.activation(
                        rzs[:, sl, :], P_all[:, 1, sl, j, :], AF.Sigmoid,
                        scale=INV_WSCALE,
                    )
                    # p = z*h (off critical path)
                    nc.gpsimd.tensor_tensor(
                        pt[:, sl, :], rzs[:, sl, :], hW[:, sl, base, :], ALU.mult
                    )
                    nc.vector.tensor_tensor(
                        rnt[:, sl, :], przt[:, sl, :], P_hn[:, sl, j, :], ALU.mult
                    )
                    nc.vector.tensor_tensor(
                        npret[:, sl, :], rnt[:, sl, :], P_all[:, 2, sl, j, :], ALU.add
                    )
                    nc.scalar.activation(
                        nt[:, sl, :], npret[:, sl, :], AF.Tanh, scale=INV_WSCALE
                    )
                    # tneg = (z - 1) * n
                    nc.vector.scalar_tensor_tensor(
                        tneg[:, sl, :], rzs[:, sl, :], 1.0, nt[:, sl, :],
                        ALU.subtract, ALU.mult,
                    )
                    # h' = p - tneg = z*h + (1-z)*n
                    nc.vector.tensor_tensor(
                        hW[:, sl, base + 1, :], pt[:, sl, :], tneg[:, sl, :],
                        ALU.subtract,
                    )

                if m + 1 < NR:
                    emit_h_ih(P_nxt, m + 1, C // 2, C)
                    P_all, P_hn = P_nxt, Ph_nxt
                    xc_cur = xc_nxt

            # final FC on last timestep of layer 4
            pfc = pfcp.tile([BC, OUT], F32, tag="fc")
            nc.tensor.matmul(
                pfc[:, :],
                hW[:, L - 1, WDIM - 1, :],
                fcT_sb[:, :],
                start=True,
                stop=True,
                skip_group_check=True,
            )
            out_sb = pp.tile([BC, OUT], F32, tag="out")
            nc.vector.tensor_tensor(out_sb[:, :], pfc[:, :], fcb_sb[:, :], ALU.add)
            nc.sync.dma_start(y[:, :], out_sb[:, :])

    nc.compile()
    return nc


def prep_shared(w_ih0, w_ih_rest, w_hh, b_ih, b_hh, fc_w, fc_b):
    d = {}
    whhT = np.empty([H, L, 3, H], np.float32)
    for l in range(L):
        for g in range(3):
            whhT[:, l, g, :] = w_hh[l, g * H : (g + 1) * H, :].T
    d["whhT"] = (whhT * WSCALE).astype(F8NP)
    wihT = np.empty([H, L - 1, 3, H], np.float32)
    for l in range(1, L):
        for g in range(3):
            wihT[:, l - 1, g, :] = w_ih_rest[l - 1, g * H : (g + 1) * H, :].T
    d["wihT"] = (wihT * WSCALE).astype(BFNP)
    wih0T = np.empty([H, 4, 3, H], np.float32)
    for ki in range(4):
        for g in range(3):
            wih0T[:, ki, g, :] = w_ih0[g * H : (g + 1) * H, ki * H : (ki + 1) * H].T
    d["wih0T"] = (wih0T * WSCALE).astype(BFNP)
    d["fcT"] = np.ascontiguousarray(fc_w.T).astype(BFNP)
    CB = C * BC  # cols per (gate, layer) combo
    biasP = np.zeros([8, 2, H], np.float32)
    for i in range(15):  # combo index i = g*5 + l
        g, l = divmod(i, 5)
        b = b_ih[l, g * H : (g + 1) * H].astype(np.float32)
        if g < 2:
            b = b + b_hh[l, g * H : (g + 1) * H]
        biasP[i % 8, i // 8, :] = b  # dims: [k, bank, H]
    d["biasP"] = (biasP * WSCALE).astype(BFNP)
    oh8 = np.zeros([8, 512], np.float32)
    for k in range(8):
        oh8[k, k * CB : (k + 1) * CB] = 1.0
    d["oh8"] = oh8.astype(BFNP)
    d["bhn5"] = (b_hh[:, 2 * H : 3 * H].astype(np.float32) * WSCALE).astype(BFNP)
    d["fcb"] = np.tile(fc_b.astype(np.float32)[None, :], (BC, 1))
    return d


_NC_CACHE = {}


def run(x, w_ih0, w_ih_rest, w_hh, b_ih, b_hh, fc_w, fc_b, T=512, **run_kwargs):
    from concourse.bass_utils import run_bass_kernel_spmd

    if T not in _NC_CACHE:
        _NC_CACHE[T] = build_nc(T)
    nc = _NC_CACHE[T]
    shared = prep_shared(
        np.asarray(w_ih0), np.asarray(w_ih_rest), np.asarray(w_hh),
        np.asarray(b_ih), np.asarray(b_hh), np.asarray(fc_w), np.asarray(fc_b),
    )
    x = np.asarray(x)
    in_maps = []
    for c in range(NCORE):
        m = dict(shared)
        m["xT"] = prep_x(x, c, T)
        in_maps.append(m)
    res = run_bass_kernel_spmd(nc, in_maps, core_ids=list(range(NCORE)), **run_kwargs)
    out = np.concatenate([res.results[c]["y"] for c in range(NCORE)], axis=0)
    return out, res


def prep_x(x, c, T):
    xs = x[c * BC : (c + 1) * BC, :T, :]  # [BC, T, IN]
    # -> [H, T, 4, BC]: element [p, t, ki, b] = x[b, t, ki*128+p]
    xt = xs.transpose(2, 1, 0).reshape(4, H, T, BC)  # [ki, p, t, b]
    return np.ascontiguousarray(xt.transpose(1, 2, 0, 3)).astype(BFNP)


def kernel(x, w_ih0, w_ih_rest, w_hh, b_ih, b_hh, fc_w, fc_b):
    out, _ = run(x, w_ih0, w_ih_rest, w_hh, b_ih, b_hh, fc_w, fc_b, T=512)
    return out.astype(np.float32)


if __name__ == "__main__":
    # quick smoke test at small T against a numpy reference
    T = int(sys.argv[1]) if len(sys.argv) > 1 else 64
    rng = np.random.default_rng(0)
    s = 1.0 / np.sqrt(H)
    u = lambda *sh: rng.uniform(-s, s, sh).astype(np.float32)
    x = rng.standard_normal((64, T, IN), dtype=np.float32)
    w_ih0 = u(3 * H, IN)
    w_ih_rest = u(L - 1, 3 * H, H)
    w_hh = u(L, 3 * H, H)
    b_ih = u(L, 3 * H)
    b_hh = u(L, 3 * H)
    fc_w = u(OUT, H)
    fc_b = u(OUT)

    def np_ref():
        sig = lambda v: 1.0 / (1.0 + np.exp(-v))
        h_in = x.astype(np.float64)
        for l in range(L):
            wi = (w_ih0 if l == 0 else w_ih_rest[l - 1]).astype(np.float64)
            wh = w_hh[l].astype(np.float64)
            gx = np.einsum("bti,gi->btg", h_in, wi) + b_ih[l]
            h = np.zeros((64, H))
            hs = []
            for t in range(T):
                gh = h @ wh.T + b_hh[l]
                xr, xz, xn = np.split(gx[:, t], 3, -1)
                hr, hz, hn = np.split(gh, 3, -1)
                r = sig(xr + hr)
                z = sig(xz + hz)
                n = np.tanh(xn + r * hn)
                h = (1 - z) * n + z * h
                hs.append(h)
            h_in = np.stack(hs, 1)
        return h_in[:, -1] @ fc_w.astype(np.float64).T + fc_b

    exp = np_ref()
    got, res = run(x, w_ih0, w_ih_rest, w_hh, b_ih, b_hh, fc_w, fc_b, T=T)
    err = np.abs(got - exp)
    rel = np.linalg.norm(got - exp) / np.linalg.norm(exp)
    print("max abs err:", err.max(), "rel:", rel)
    print("exec_time_ns:", res.exec_time_ns)
